# revision 11
# baseline (speedup 1.0000x reference)
"""GAT BasicAttentionBlock kernel for 8x Trainium2 NeuronCores.

Strategy (output-shard, v2): each core owns 1250 of the 10000 selected
output rows (index0).  Only nodes reachable from those rows matter
(~1.2k targets + ~16k sources per core).  Per core:

  node table order: [window-grouped targets (1280 rows) | sources sorted
  by per-core edge multiplicity desc].  A static row boundary B1 (mult
  of 512) splits the table so rows < B1 carry ~2/3 of the edges.

  phase A  stream x columns of the table nodes (bf16), h = relu(x@w1.T)
           feature-major on PE, then per 128-node subtile
           proj|s_src = h@w2 node-major; copy cols 0:136 (bf16) of each
           PSUM tile to SBUF and DMA full 512-byte rows to the HBM table.
           Emission of loop-1 work (s_trg/skip/one-hot masks) is
           interleaved into phase A's engine slack.
  gathers  per 128-target window, edges sorted by source row: slots
           [0,KLO) hold only sources < B1 and are gathered as soon as
           the lo part of the table is written (overlaps phase A);
           slots [KLO,EC) gather after the full table.
  loop 2   per window: scores = lrelu(s_src+s_trg) on ACT (alpha=.2),
           e = exp, weighted = e*proj, segment-sum via one-hot matmuls
           accumulated in PSUM [sum e*proj | sum e]; out = att/den +
           skip, ELU; windows finish staggered as hi-gathers land.
  final    dma_gather the 1250 output rows from the 1280-target table.

No collectives: cores are fully independent.  The softmax global max
subtraction cancels in att = exp/sum(exp) and is dropped.
"""

import os
import sys

for _p in ("/opt/trn_rl_repo",):
    if os.path.isdir(_p) and _p not in sys.path:
        sys.path.insert(0, _p)

import numpy as np
import ml_dtypes

# problem constants (hardcoded per contract)
N = 50000
E = 800000
K = 10000
IN = 256
H = 128
NH = 8
HD = 16
OC = NH * HD  # 128
CORES = 8
KC = K // CORES          # 1250 output rows per core
P = 128
W = 10                   # target windows of 128 -> 1280 target slots
TP = W * P               # padded target count per core
EPS = 1e-16

BF16 = ml_dtypes.bfloat16

LOFRAC = float(os.environ.get("KLOFRAC", "0.70"))


# ----------------------------------------------------------------------------
# host-side sharding / planning
# ----------------------------------------------------------------------------

def _wrap16(vals, reps=8):
    """int16 index layout for dma_gather: idx i at [i%16, i//16], the 16-row
    block replicated `reps` times down the partition axis."""
    L = vals.shape[0]
    assert L % 16 == 0
    w = vals.reshape(L // 16, 16).T.astype(np.int16)
    return np.tile(w, (reps, 1))


def _binpack(deg):
    """Assign targets (by degree desc) to W windows (<=128 each), balancing
    total degree.  Returns row index (w*128 + pos) per target."""
    U = len(deg)
    order = np.argsort(-deg, kind="stable")
    wdeg = np.zeros(W)
    wcnt = np.zeros(W, np.int64)
    row = np.zeros(U, np.int64)
    for u in order:
        cand = np.nonzero(wcnt < P)[0]
        wsel = cand[np.argmin(wdeg[cand])]
        row[u] = wsel * P + wcnt[wsel]
        wcnt[wsel] += 1
        wdeg[wsel] += deg[u]
    return row


def plan(x, adj0, index0):
    src_all = np.asarray(adj0[0], dtype=np.int64)
    trg_all = np.asarray(adj0[1], dtype=np.int64)
    idx0 = np.asarray(index0, dtype=np.int64)
    x = np.asarray(x, dtype=np.float32)

    pre = []
    npad_req = 512
    for c in range(CORES):
        ks = idx0[c * KC:(c + 1) * KC]
        tgt_u, inv_k = np.unique(ks, return_inverse=True)
        U_t = len(tgt_u)
        assert U_t <= TP
        lut = np.full(N, -1, np.int64)
        lut[tgt_u] = np.arange(U_t)
        tloc_all = lut[trg_all]
        sel = np.nonzero(tloc_all >= 0)[0]
        e_src = src_all[sel]
        e_tu = tloc_all[sel]
        deg = np.bincount(e_tu, minlength=U_t)
        trow = _binpack(deg)                       # tgt_u idx -> table row

        # source rows: targets keep their rows; extras sorted by edge count
        nrow = np.full(N, -1, np.int64)
        nrow[tgt_u] = trow
        is_extra = nrow[e_src] < 0
        ex_ids, ex_cnt_inv = np.unique(e_src[is_extra], return_inverse=True)
        ex_cnt = np.bincount(ex_cnt_inv)
        ex_order = np.argsort(-ex_cnt, kind="stable")
        extras = ex_ids[ex_order]
        nrow[extras] = TP + np.arange(len(extras))
        U_n = TP + len(extras)
        npad_req = max(npad_req, U_n)

        e_srow = nrow[e_src]                       # source table row per edge
        e_trow = trow[e_tu]                        # target table row per edge
        # node id per table row (for xT); pad rows -> x of node 0 (harmless)
        nodes = np.zeros(U_n, np.int64)
        nodes[trow] = tgt_u
        nodes[TP:] = extras
        pre.append((trow, inv_k, e_srow, e_trow, nodes, U_n))

    NPAD = ((npad_req + 511) // 512) * 512

    # shared lo-chunk boundary B1 (mult of 512): rows < B1 carry >= LOFRAC
    # of edges on every core
    b1_req = 512
    for c in range(CORES):
        _, _, e_srow, _, _, _ = pre[c]
        hist = np.bincount(e_srow // 512, minlength=NPAD // 512)
        cum = np.cumsum(hist) / len(e_srow)
        t = int(np.searchsorted(cum, LOFRAC)) + 1
        b1_req = max(b1_req, t * 512)
    B1 = min(b1_req, NPAD)

    # per-core, per-window edge packing: sort by source row (lo first)
    ec_req = 1
    klo_min = 10**9
    packed = []
    for c in range(CORES):
        trow, inv_k, e_srow, e_trow, nodes, U_n = pre[c]
        e_win = e_trow >> 7
        order = np.lexsort((e_srow, e_win))
        e_srow = e_srow[order]
        e_trow = e_trow[order]
        e_win = e_win[order]
        cnt = np.bincount(e_win, minlength=W)
        nlo = np.bincount(e_win[e_srow < B1], minlength=W)
        ec_req = max(ec_req, int(np.ceil(cnt.max() / P)))
        klo_min = min(klo_min, int(nlo.min()) // P)
        packed.append((trow, inv_k, e_srow, e_trow, e_win, cnt, nodes))

    EC = ec_req
    KLO = min(klo_min, EC)
    assert 1 <= KLO < EC, (KLO, EC)
    cap = EC * P

    per_core = []
    for c in range(CORES):
        trow, inv_k, e_srow, e_trow, e_win, cnt, nodes = packed[c]
        start = np.concatenate([[0], np.cumsum(cnt)[:-1]])
        within = np.arange(len(e_trow)) - start[e_win]
        slots = e_win * cap + within

        esrc_flat = np.zeros(W * cap, np.int64)
        etcol_flat = np.full(W * cap, -1.0, np.float32)
        esrc_flat[slots] = e_srow
        etcol_flat[slots] = (e_trow - e_win * P).astype(np.float32)

        etcol = etcol_flat.reshape(W, EC, P).transpose(2, 0, 1).reshape(P, W * EC)
        etrow_b = etcol_flat.astype(BF16).reshape(1, W * cap)
        eidx_lo = np.concatenate(
            [_wrap16(esrc_flat[w * cap:w * cap + KLO * P]) for w in range(W)],
            axis=1)
        eidx_hi = np.concatenate(
            [_wrap16(esrc_flat[w * cap + KLO * P:(w + 1) * cap])
             for w in range(W)], axis=1)

        kvals = np.zeros(TP, np.int64)
        kvals[:KC] = trow[inv_k]
        kidx = _wrap16(kvals)

        xT = np.zeros((IN, NPAD), BF16)
        xT[:, :len(nodes)] = x[nodes].T

        per_core.append(dict(xT=xT, eidx_lo=eidx_lo, eidx_hi=eidx_hi,
                             etcol=etcol, etrow=etrow_b, kidx=kidx))
    return per_core, NPAD, EC, KLO, B1


def make_weights(w_in, b_in, w_proj, a_src, a_trg, w_skip):
    w_in = np.asarray(w_in, np.float32)
    b_in = np.asarray(b_in, np.float32)
    w_proj = np.asarray(w_proj, np.float32)
    a_src = np.asarray(a_src, np.float32).reshape(NH, HD)
    a_trg = np.asarray(a_trg, np.float32).reshape(NH, HD)
    w_skip = np.asarray(w_skip, np.float32)

    w1T = np.ascontiguousarray(w_in.T).astype(BF16)        # [256,128]
    b1 = b_in.reshape(H, 1).astype(np.float32)
    # B_src[h, a] = sum_d w_proj[a*16+d, h] * a_src[a, d]
    wp3 = w_proj.reshape(NH, HD, H)
    B_src = np.einsum("adh,ad->ha", wp3, a_src).astype(np.float32)  # [128,8]
    B_trg = np.einsum("adh,ad->ha", wp3, a_trg).astype(BF16)
    w2 = np.zeros((H, 256), np.float32)
    w2[:, :OC] = w_proj.T
    w2[:, OC:OC + NH] = B_src
    wskT = np.ascontiguousarray(w_skip.T).astype(BF16)     # [128,128]
    iota4 = np.tile(np.arange(P, dtype=BF16)[None, :], (P, 2))
    iota_c = np.arange(P, dtype=np.float32).reshape(P, 1)
    return dict(w1T=w1T, b1=b1, w2=w2, wskT=wskT, btrg=B_trg,
                iota4=iota4, iota_c=iota_c)


# ----------------------------------------------------------------------------
# bass kernel
# ----------------------------------------------------------------------------

_BUILD_CACHE = {}


def build(NPAD, EC, KLO, B1):
    key = (NPAD, EC, KLO, B1)
    if key in _BUILD_CACHE:
        return _BUILD_CACHE[key]

    import concourse.bacc as bacc
    import concourse.mybir as mybir
    import concourse.tile as tile

    dt = mybir.dt
    F32 = dt.float32
    F32R = dt.float32r
    I16 = dt.int16
    BF = dt.bfloat16
    AF = mybir.ActivationFunctionType
    OP = mybir.AluOpType

    NT = NPAD // 512
    cap = EC * P
    KHI = EC - KLO

    nc = bacc.Bacc("TRN2", target_bir_lowering=False)

    with tile.TileContext(nc) as tc:
        with tc.tile_pool(name="dram", bufs=1, space="DRAM") as dram:
            def din(name, shape, dtp):
                return dram.tile(shape, dtp, kind="ExternalInput", name=name,
                                 uniquify=False)

            xT = din("xT", [IN, NPAD], BF)
            w1T = din("w1T", [IN, H], BF)
            b1 = din("b1", [H, 1], F32)
            w2 = din("w2", [H, 256], F32R)
            wskT = din("wskT", [H, OC], BF)
            btrg = din("btrg", [H, NH], BF)
            eidx_lo = din("eidx_lo", [P, W * KLO * 8], I16)
            eidx_hi = din("eidx_hi", [P, W * KHI * 8], I16)
            etcol = din("etcol", [P, W * EC], F32)
            etrow = din("etrow", [1, W * cap], BF)
            kidx = din("kidx", [P, TP // 16], I16)
            iota4 = din("iota4", [P, 2 * P], BF)
            iota_c = din("iota_c", [P, 1], F32)

            tabA = dram.tile([NPAD, 256], BF, kind="Internal", name="tabA",
                             uniquify=False)
            outT = dram.tile([TP, OC], BF, kind="Internal", name="outT",
                             uniquify=False)
            out = dram.tile([TP, OC], BF, kind="ExternalOutput", name="out",
                            uniquify=False)

        with tc.tile_pool(name="pers", bufs=1) as pers:
            w1a = pers.tile([P, H], BF)
            w1b = pers.tile([P, H], BF)
            b1s = pers.tile([H, 1], F32)
            w2s = pers.tile([H, 256], F32R)
            wsks = pers.tile([H, OC], BF)
            btrgs = pers.tile([H, NH], BF)
            iota4s = pers.tile([P, 2 * P], BF)
            iotac = pers.tile([P, 1], F32)
            hfmt = pers.tile([H, TP], BF)         # targets' h, feature-major
            strg = pers.tile([P, W * NH], BF)     # per-window s_trg  [t, 8]
            skips = pers.tile([P, W, OC], BF)     # per-window skip   [t, oc]
            st_sb = pers.tile([P, W, EC, NH], BF)   # s_trg per edge slot
            Mw = pers.tile([P, W * cap], BF)      # edge->target one-hot
            Glo = pers.tile([P, W, KLO, 256], BF)  # lo-gathered table rows
            eloidx = pers.tile([P, W * KLO * 8], I16)
            ehiidx = pers.tile([P, W * KHI * 8], I16)
            etcols = pers.tile([P, W * EC], F32)
            kidxs = pers.tile([P, TP // 16], I16)
            etws = pers.tile([1, W * cap], BF)

            nc.sync.dma_start(w1a[:], w1T[0:P, :])
            nc.sync.dma_start(w1b[:], w1T[P:IN, :])
            nc.sync.dma_start(b1s[:], b1[:])
            nc.sync.dma_start(w2s[:], w2[:])
            nc.sync.dma_start(wsks[:], wskT[:])
            nc.sync.dma_start(btrgs[:], btrg[:])
            nc.sync.dma_start(iota4s[:], iota4[:])
            nc.sync.dma_start(iotac[:], iota_c[:])
            nc.sync.dma_start(eloidx[:], eidx_lo[:])
            nc.sync.dma_start(ehiidx[:], eidx_hi[:])
            nc.sync.dma_start(etcols[:], etcol[:])
            nc.sync.dma_start(kidxs[:], kidx[:])
            nc.sync.dma_start(etws[:], etrow[:])

            CH = 2  # 512-node tiles per xT load chunk
            with tc.tile_pool(name="pa", bufs=2) as pa, \
                 tc.tile_pool(name="pax", bufs=2) as pax, \
                 tc.tile_pool(name="pbc", bufs=2) as pbc, \
                 tc.tile_pool(name="pmtw", bufs=2) as pmtw, \
                 tc.tile_pool(name="pghi", bufs=2) as pghi, \
                 tc.tile_pool(name="pe2", bufs=2) as pe2, \
                 tc.tile_pool(name="pko", bufs=1) as pko, \
                 tc.tile_pool(name="psa", bufs=2, space="PSUM") as psa, \
                 tc.tile_pool(name="psb", bufs=2, space="PSUM") as psb, \
                 tc.tile_pool(name="psc", bufs=1, space="PSUM") as psc, \
                 tc.tile_pool(name="psd", bufs=1, space="PSUM") as psd, \
                 tc.tile_pool(name="pse", bufs=2, space="PSUM") as pse:

                # ---- partition-broadcast of per-slot target cols (Pool) ----
                pbcs = []
                for w in range(W):
                    pbcw = pbc.tile([P, cap], BF, tag="pbcw")
                    nc.gpsimd.partition_broadcast(
                        pbcw[:], etws[0:1, w * cap:(w + 1) * cap])
                    pbcs.append(pbcw)

                # deferred emissions interleaved into phase A slack
                def emit_loop1(w):
                    # s_trg / skip for the window targets
                    stp = psd.tile([P, OC], F32, tag="misc")
                    nc.tensor.matmul(stp[:, 0:NH],
                                     lhsT=hfmt[:, w * P:(w + 1) * P],
                                     rhs=btrgs[:], start=True, stop=True)
                    nc.vector.tensor_copy(strg[:, w * NH:(w + 1) * NH],
                                          stp[:, 0:NH])
                    skp = psd.tile([P, OC], F32, tag="misc")
                    nc.tensor.matmul(skp[:], lhsT=hfmt[:, w * P:(w + 1) * P],
                                     rhs=wsks[:], start=True, stop=True)
                    nc.vector.tensor_copy(skips[:, w], skp[:])
                    # target one-hot (col-major) and s_trg edge-slot expansion
                    Mtw = pmtw.tile([P, cap], BF, tag="Mtw")
                    nc.vector.tensor_scalar(Mtw[:], pbcs[w][:], iotac[:], None,
                                            OP.is_equal)
                    stps = psc.tile([P, EC, NH], F32, tag="stps")
                    for j in range(EC):
                        nc.tensor.matmul(
                            stps[:, j, :], lhsT=Mtw[:, j * P:(j + 1) * P],
                            rhs=strg[:, w * NH:(w + 1) * NH],
                            start=True, stop=True)
                    nc.vector.tensor_copy(st_sb[:, w], stps[:])

                def emit_mw(w, j):
                    col = w * EC + j
                    nc.vector.tensor_scalar(
                        Mw[:, col * P:(col + 1) * P], iota4s[:, 0:P],
                        etcols[:, col:col + 1], None, OP.is_equal)

                # schedule: loop1(w) at tile 2+w; Mw slots spread over tiles
                mw_jobs = [(w, j) for w in range(W) for j in range(EC)]
                mw_t0, mw_t1 = 3, NT - 1
                def mw_share(t):
                    if t < mw_t0:
                        return []
                    a = len(mw_jobs) * (t - mw_t0) // (mw_t1 - mw_t0)
                    b = len(mw_jobs) * (t + 1 - mw_t0) // (mw_t1 - mw_t0)
                    return mw_jobs[a:b]

                # ---------------- phase A ----------------
                for t0 in range(0, NT, CH):
                    t1 = min(t0 + CH, NT)
                    wdc = (t1 - t0) * 512
                    slc = slice(t0 * 512, t0 * 512 + wdc)
                    xa = pax.tile([P, CH * 512], BF, tag="xa")
                    nc.sync.dma_start(xa[:, 0:wdc], xT[0:P, slc])
                    xb = pax.tile([P, CH * 512], BF, tag="xb")
                    nc.sync.dma_start(xb[:, 0:wdc], xT[P:IN, slc])
                    for t in range(t0, t1):
                        o = (t - t0) * 512
                        hps = psa.tile([P, 512], F32, tag="hps")
                        nc.tensor.matmul(hps[:], lhsT=w1a[:],
                                         rhs=xa[:, o:o + 512],
                                         start=True, stop=False)
                        nc.tensor.matmul(hps[:], lhsT=w1b[:],
                                         rhs=xb[:, o:o + 512],
                                         start=False, stop=True)
                        hsb = pa.tile([P, 512], F32R, tag="hsb")
                        nc.scalar.activation(hsb[:], hps[:], AF.Relu,
                                             bias=b1s[:])
                        if t * 512 < TP:
                            w0 = t * 512
                            w1_ = min(TP, (t + 1) * 512)
                            nc.scalar.activation(hfmt[:, w0:w1_],
                                                 hps[:, 0:(w1_ - w0)], AF.Relu,
                                                 bias=b1s[:])
                        stg = pa.tile([P, 4, 256], BF, tag="stg")
                        for half in range(2):
                            p2 = psb.tile([P, 2, 256], F32, tag="p2")
                            for jj in range(2):
                                j = half * 2 + jj
                                nc.tensor.matmul(
                                    p2[:, jj, :],
                                    lhsT=hsb[:, j * P:(j + 1) * P],
                                    rhs=w2s[:], start=True, stop=True)
                            sgh = stg[:, half * 2:half * 2 + 2, :]
                            if half == 0:
                                nc.scalar.activation(sgh[:, :, 0:OC + NH],
                                                     p2[:, :, 0:OC + NH],
                                                     AF.Copy)
                            else:
                                nc.vector.tensor_copy(sgh[:, :, 0:OC + NH],
                                                      p2[:, :, 0:OC + NH])
                        r0 = t * 512
                        nc.sync.dma_start(
                            tabA[r0:r0 + 512, :].rearrange(
                                "(j p) f -> p j f", p=P), stg[:])
                        # interleaved loop-1 / mask emissions
                        if 2 <= t < 2 + W:
                            emit_loop1(t - 2)
                        for (w_, j_) in mw_share(t):
                            emit_mw(w_, j_)

                # ---------------- gathers ----------------
                # lo gathers read only rows [0, B1): they fire as soon as
                # the lo part of the table is written (overlaps phase A)
                for w in range(W):
                    nc.gpsimd.dma_gather(
                        Glo[:, w], tabA[0:B1],
                        eloidx[:, w * KLO * 8:(w + 1) * KLO * 8],
                        KLO * P, KLO * P, 256, single_packet=False)
                ghis = []
                for w in range(W):
                    G = pghi.tile([P, KHI, 256], BF, tag="G")
                    nc.gpsimd.dma_gather(
                        G[:], tabA[:],
                        ehiidx[:, w * KHI * 8:(w + 1) * KHI * 8],
                        KHI * P, KHI * P, 256, single_packet=False)
                    ghis.append(G)

                # ---------------- loop 2: per-window edge pipeline ----------
                def finalize(w, segp):
                    den = pe2.tile([P, NH], F32, tag="den")
                    nc.vector.tensor_scalar_add(den[:], segp[:, OC:OC + NH],
                                                EPS)
                    rec = pe2.tile([P, NH], F32, tag="rec")
                    nc.vector.reciprocal(rec[:], den[:])
                    z = pe2.tile([P, OC], F32, tag="z")
                    recb = rec[:].broadcast_to([P, NH, HD])
                    nc.vector.tensor_tensor(
                        z[:].rearrange("p (a d) -> p a d", d=HD),
                        segp[:, 0:OC].rearrange("p (a d) -> p a d", d=HD),
                        recb, OP.mult)
                    nc.vector.tensor_add(z[:], z[:], skips[:, w])
                    # elu: (max(z,0)-1) + exp(min(z,0))
                    am = pe2.tile([P, OC], F32, tag="am")
                    nc.vector.tensor_scalar(am[:], z[:], 0.0, -1.0, OP.max,
                                            OP.add)
                    bm = pe2.tile([P, OC], F32, tag="bm")
                    nc.gpsimd.tensor_scalar(bm[:], z[:], 0.0, None, OP.min)
                    eb = pe2.tile([P, OC], F32, tag="eb")
                    nc.scalar.activation(eb[:], bm[:], AF.Exp)
                    fo = pe2.tile([P, OC], BF, tag="fo")
                    nc.vector.tensor_add(fo[:], am[:], eb[:])
                    nc.sync.dma_start(outT[w * P:(w + 1) * P, :], fo[:])

                pending = None
                for w in range(W):
                    G = ghis[w]
                    # scores = s_src(gathered) + s_trg(expanded)
                    sc = pe2.tile([P, EC, NH], F32, tag="sc")
                    glo_ss = Glo[:, w, :, OC:OC + NH]
                    nc.vector.tensor_tensor(sc[:, 0:KLO], st_sb[:, w, 0:KLO],
                                            glo_ss, OP.add)
                    if KHI:
                        nc.vector.tensor_tensor(sc[:, KLO:EC],
                                                st_sb[:, w, KLO:EC],
                                                G[:, :, OC:OC + NH], OP.add)
                    # e = exp(leakyrelu(s, 0.2))
                    el = pe2.tile([P, EC, NH], F32, tag="el")
                    nc.scalar.activation(el[:], sc[:], AF.Lrelu, alpha=0.2)
                    emax = pe2.tile([P, EC, NH], BF, tag="emax")
                    nc.scalar.activation(emax[:], el[:], AF.Exp)
                    Wv = pe2.tile([P, EC, 136], BF, tag="Wv")
                    nc.vector.tensor_copy(Wv[:, :, OC:OC + NH], emax[:])
                    # expand e per-head on ACT (broadcast read), then a
                    # fully packed bf16 multiply on DVE (2x mode)
                    eex = pe2.tile([P, EC, OC], BF, tag="eex")
                    nc.scalar.activation(
                        eex[:].rearrange("p j (a d) -> p j a d", d=HD),
                        emax[:].broadcast_to([P, EC, NH, HD]), AF.Copy)
                    nc.vector.tensor_tensor(Wv[:, 0:KLO, 0:OC],
                                            Glo[:, w, :, 0:OC],
                                            eex[:, 0:KLO], OP.mult)
                    if KHI:
                        nc.vector.tensor_tensor(Wv[:, KLO:EC, 0:OC],
                                                G[:, :, 0:OC],
                                                eex[:, KLO:EC], OP.mult)

                    segp = pse.tile([P, 136], F32, tag="segp")
                    for j in range(EC):
                        nc.tensor.matmul(segp[:],
                                         lhsT=Mw[:, (w * EC + j) * P:
                                                 (w * EC + j + 1) * P],
                                         rhs=Wv[:, j, :], start=(j == 0),
                                         stop=(j == EC - 1))
                    if pending is not None:
                        finalize(*pending)
                    pending = (w, segp)
                if pending is not None:
                    finalize(*pending)

                # final k-row gather
                ko = pko.tile([P, TP // P, OC], BF, tag="ko")
                nc.gpsimd.dma_gather(ko[:], outT[:], kidxs[:], TP, TP, OC,
                                     single_packet=False)
                nc.sync.dma_start(
                    out[:].rearrange("(j p) f -> p j f", p=P), ko[:])

    nc.compile()
    _BUILD_CACHE[key] = nc
    return nc


# ----------------------------------------------------------------------------
# entry point
# ----------------------------------------------------------------------------

def kernel(x, adj0, index0, w_in, b_in, w_proj, a_src, a_trg, w_skip):
    from concourse.bass_utils import run_bass_kernel_spmd

    per_core, NPAD, EC, KLO, B1 = plan(x, adj0, index0)
    wts = make_weights(w_in, b_in, w_proj, a_src, a_trg, w_skip)
    nc = build(NPAD, EC, KLO, B1)

    in_maps = []
    for c in range(CORES):
        m = dict(wts)
        m.update(per_core[c])
        in_maps.append(m)

    res = run_bass_kernel_spmd(nc, in_maps, core_ids=list(range(CORES)))
    outs = [r["out"][:KC] for r in res.results]
    return np.concatenate(outs, axis=0).astype(np.float32)


# revision 13
# speedup vs baseline: 1.1886x; 1.1886x over previous
"""GAT BasicAttentionBlock kernel for 8x Trainium2 NeuronCores.

Strategy (output-shard, v2): each core owns 1250 of the 10000 selected
output rows (index0).  Only nodes reachable from those rows matter
(~1.2k targets + ~16k sources per core).  Per core:

  node table order: [window-grouped targets (1280 rows) | sources sorted
  by per-core edge multiplicity desc].  A static row boundary B1 (mult
  of 512) splits the table so rows < B1 carry ~2/3 of the edges.

  phase A  stream x columns of the table nodes (bf16), h = relu(x@w1.T)
           feature-major on PE, then per 128-node subtile
           proj|s_src = h@w2 node-major; copy cols 0:136 (bf16) of each
           PSUM tile to SBUF and DMA full 512-byte rows to the HBM table.
           Emission of loop-1 work (s_trg/skip/one-hot masks) is
           interleaved into phase A's engine slack.
  gathers  per 128-target window, edges sorted by source row: slots
           [0,KLO) hold only sources < B1 and are gathered as soon as
           the lo part of the table is written (overlaps phase A);
           slots [KLO,EC) gather after the full table.
  loop 2   per window: scores = lrelu(s_src+s_trg) on ACT (alpha=.2),
           e = exp, weighted = e*proj, segment-sum via one-hot matmuls
           accumulated in PSUM [sum e*proj | sum e]; out = att/den +
           skip, ELU; windows finish staggered as hi-gathers land.
  final    dma_gather the 1250 output rows from the 1280-target table.

No collectives: cores are fully independent.  The softmax global max
subtraction cancels in att = exp/sum(exp) and is dropped.
"""

import os
import sys

for _p in ("/opt/trn_rl_repo",):
    if os.path.isdir(_p) and _p not in sys.path:
        sys.path.insert(0, _p)

import numpy as np
import ml_dtypes

# problem constants (hardcoded per contract)
N = 50000
E = 800000
K = 10000
IN = 256
H = 128
NH = 8
HD = 16
OC = NH * HD  # 128
CORES = 8
KC = K // CORES          # 1250 output rows per core
P = 128
W = 10                   # target windows of 128 -> 1280 target slots
TP = W * P               # padded target count per core
EPS = 1e-16

BF16 = ml_dtypes.bfloat16

LOFRAC = float(os.environ.get("KLOFRAC", "0.70"))


# ----------------------------------------------------------------------------
# host-side sharding / planning
# ----------------------------------------------------------------------------

def _wrap16(vals, reps=8):
    """int16 index layout for dma_gather: idx i at [i%16, i//16], the 16-row
    block replicated `reps` times down the partition axis."""
    L = vals.shape[0]
    assert L % 16 == 0
    w = vals.reshape(L // 16, 16).T.astype(np.int16)
    return np.tile(w, (reps, 1))


def _binpack(deg):
    """Assign targets (by degree desc) to W windows (<=128 each), balancing
    total degree.  Returns row index (w*128 + pos) per target."""
    U = len(deg)
    order = np.argsort(-deg, kind="stable")
    wdeg = np.zeros(W)
    wcnt = np.zeros(W, np.int64)
    row = np.zeros(U, np.int64)
    for u in order:
        cand = np.nonzero(wcnt < P)[0]
        wsel = cand[np.argmin(wdeg[cand])]
        row[u] = wsel * P + wcnt[wsel]
        wcnt[wsel] += 1
        wdeg[wsel] += deg[u]
    return row


def plan(x, adj0, index0):
    src_all = np.asarray(adj0[0], dtype=np.int64)
    trg_all = np.asarray(adj0[1], dtype=np.int64)
    idx0 = np.asarray(index0, dtype=np.int64)
    x = np.asarray(x, dtype=np.float32)

    pre = []
    npad_req = 512
    for c in range(CORES):
        ks = idx0[c * KC:(c + 1) * KC]
        tgt_u, inv_k = np.unique(ks, return_inverse=True)
        U_t = len(tgt_u)
        assert U_t <= TP
        lut = np.full(N, -1, np.int64)
        lut[tgt_u] = np.arange(U_t)
        tloc_all = lut[trg_all]
        sel = np.nonzero(tloc_all >= 0)[0]
        e_src = src_all[sel]
        e_tu = tloc_all[sel]
        deg = np.bincount(e_tu, minlength=U_t)
        trow = _binpack(deg)                       # tgt_u idx -> table row

        # source rows: targets keep their rows; extras sorted by edge count
        nrow = np.full(N, -1, np.int64)
        nrow[tgt_u] = trow
        is_extra = nrow[e_src] < 0
        ex_ids, ex_cnt_inv = np.unique(e_src[is_extra], return_inverse=True)
        ex_cnt = np.bincount(ex_cnt_inv)
        ex_order = np.argsort(-ex_cnt, kind="stable")
        extras = ex_ids[ex_order]
        nrow[extras] = TP + np.arange(len(extras))
        U_n = TP + len(extras)
        npad_req = max(npad_req, U_n)

        e_srow = nrow[e_src]                       # source table row per edge
        e_trow = trow[e_tu]                        # target table row per edge
        # node id per table row (for xT); pad rows -> x of node 0 (harmless)
        nodes = np.zeros(U_n, np.int64)
        nodes[trow] = tgt_u
        nodes[TP:] = extras
        pre.append((trow, inv_k, e_srow, e_trow, nodes, U_n))

    NPAD = ((npad_req + 511) // 512) * 512

    # shared lo-chunk boundary B1 (mult of 512): rows < B1 carry >= LOFRAC
    # of edges on every core
    b1_req = 512
    for c in range(CORES):
        _, _, e_srow, _, _, _ = pre[c]
        hist = np.bincount(e_srow // 512, minlength=NPAD // 512)
        cum = np.cumsum(hist) / len(e_srow)
        t = int(np.searchsorted(cum, LOFRAC)) + 1
        b1_req = max(b1_req, t * 512)
    B1 = min(b1_req, NPAD)

    # per-core, per-window edge packing: sort by source row (lo first)
    ec_req = 1
    klo_min = 10**9
    packed = []
    for c in range(CORES):
        trow, inv_k, e_srow, e_trow, nodes, U_n = pre[c]
        e_win = e_trow >> 7
        order = np.lexsort((e_srow, e_win))
        e_srow = e_srow[order]
        e_trow = e_trow[order]
        e_win = e_win[order]
        cnt = np.bincount(e_win, minlength=W)
        nlo = np.bincount(e_win[e_srow < B1], minlength=W)
        ec_req = max(ec_req, int(np.ceil(cnt.max() / P)))
        klo_min = min(klo_min, int(nlo.min()) // P)
        packed.append((trow, inv_k, e_srow, e_trow, e_win, cnt, nodes))

    EC = ec_req
    KLO = min(klo_min, EC)
    assert 1 <= KLO < EC, (KLO, EC)
    cap = EC * P

    per_core = []
    for c in range(CORES):
        trow, inv_k, e_srow, e_trow, e_win, cnt, nodes = packed[c]
        start = np.concatenate([[0], np.cumsum(cnt)[:-1]])
        within = np.arange(len(e_trow)) - start[e_win]
        slots = e_win * cap + within

        esrc_flat = np.zeros(W * cap, np.int64)
        etcol_flat = np.full(W * cap, -1.0, np.float32)
        esrc_flat[slots] = e_srow
        etcol_flat[slots] = (e_trow - e_win * P).astype(np.float32)

        etcol = etcol_flat.reshape(W, EC, P).transpose(2, 0, 1).reshape(P, W * EC)
        etrow_b = etcol_flat.astype(BF16).reshape(1, W * cap)
        eidx_lo = np.concatenate(
            [_wrap16(esrc_flat[w * cap:w * cap + KLO * P]) for w in range(W)],
            axis=1)
        eidx_hi = np.concatenate(
            [_wrap16(esrc_flat[w * cap + KLO * P:(w + 1) * cap])
             for w in range(W)], axis=1)

        kvals = np.zeros(TP, np.int64)
        kvals[:KC] = trow[inv_k]
        kidx = _wrap16(kvals)

        xT = np.zeros((IN, NPAD), BF16)
        xT[:, :len(nodes)] = x[nodes].T

        per_core.append(dict(xT=xT, eidx_lo=eidx_lo, eidx_hi=eidx_hi,
                             etcol=etcol, etrow=etrow_b, kidx=kidx))
    return per_core, NPAD, EC, KLO, B1


def make_weights(w_in, b_in, w_proj, a_src, a_trg, w_skip):
    w_in = np.asarray(w_in, np.float32)
    b_in = np.asarray(b_in, np.float32)
    w_proj = np.asarray(w_proj, np.float32)
    a_src = np.asarray(a_src, np.float32).reshape(NH, HD)
    a_trg = np.asarray(a_trg, np.float32).reshape(NH, HD)
    w_skip = np.asarray(w_skip, np.float32)

    w1T = np.ascontiguousarray(w_in.T).astype(BF16)        # [256,128]
    b1 = b_in.reshape(H, 1).astype(np.float32)
    # B_src[h, a] = sum_d w_proj[a*16+d, h] * a_src[a, d]
    wp3 = w_proj.reshape(NH, HD, H)
    B_src = np.einsum("adh,ad->ha", wp3, a_src).astype(np.float32)  # [128,8]
    B_trg = np.einsum("adh,ad->ha", wp3, a_trg).astype(BF16)
    w2 = np.zeros((H, 256), np.float32)
    w2[:, :OC] = w_proj.T
    w2[:, OC:OC + NH] = B_src
    wskT = np.ascontiguousarray(w_skip.T).astype(BF16)     # [128,128]
    iota4 = np.tile(np.arange(P, dtype=BF16)[None, :], (P, 2))
    iota_c = np.arange(P, dtype=np.float32).reshape(P, 1)
    return dict(w1T=w1T, b1=b1, w2=w2, wskT=wskT, btrg=B_trg,
                iota4=iota4, iota_c=iota_c)


# ----------------------------------------------------------------------------
# bass kernel
# ----------------------------------------------------------------------------

_BUILD_CACHE = {}


def build(NPAD, EC, KLO, B1):
    key = (NPAD, EC, KLO, B1)
    if key in _BUILD_CACHE:
        return _BUILD_CACHE[key]

    import concourse.bacc as bacc
    import concourse.mybir as mybir
    import concourse.tile as tile

    dt = mybir.dt
    F32 = dt.float32
    F32R = dt.float32r
    I16 = dt.int16
    BF = dt.bfloat16
    AF = mybir.ActivationFunctionType
    OP = mybir.AluOpType

    NT = NPAD // 512
    cap = EC * P
    KHI = EC - KLO

    nc = bacc.Bacc("TRN2", target_bir_lowering=False)

    with tile.TileContext(nc) as tc:
        with tc.tile_pool(name="dram", bufs=1, space="DRAM") as dram:
            def din(name, shape, dtp):
                return dram.tile(shape, dtp, kind="ExternalInput", name=name,
                                 uniquify=False)

            xT = din("xT", [IN, NPAD], BF)
            w1T = din("w1T", [IN, H], BF)
            b1 = din("b1", [H, 1], F32)
            w2 = din("w2", [H, 256], F32R)
            wskT = din("wskT", [H, OC], BF)
            btrg = din("btrg", [H, NH], BF)
            eidx_lo = din("eidx_lo", [P, W * KLO * 8], I16)
            eidx_hi = din("eidx_hi", [P, W * KHI * 8], I16)
            etcol = din("etcol", [P, W * EC], F32)
            etrow = din("etrow", [1, W * cap], BF)
            kidx = din("kidx", [P, TP // 16], I16)
            iota4 = din("iota4", [P, 2 * P], BF)
            iota_c = din("iota_c", [P, 1], F32)

            tabA = dram.tile([NPAD, 256], BF, kind="Internal", name="tabA",
                             uniquify=False)
            outT = dram.tile([TP, OC], BF, kind="Internal", name="outT",
                             uniquify=False)
            out = dram.tile([TP, OC], BF, kind="ExternalOutput", name="out",
                            uniquify=False)

        with tc.tile_pool(name="pers", bufs=1) as pers:
            w1a = pers.tile([P, H], BF)
            w1b = pers.tile([P, H], BF)
            b1s = pers.tile([H, 1], F32)
            w2s = pers.tile([H, 256], F32R)
            wsks = pers.tile([H, OC], BF)
            btrgs = pers.tile([H, NH], BF)
            iota4s = pers.tile([P, 2 * P], BF)
            iotac = pers.tile([P, 1], F32)
            hfmt = pers.tile([H, TP], BF)         # targets' h, feature-major
            strg = pers.tile([P, W * NH], BF)     # per-window s_trg  [t, 8]
            skips = pers.tile([P, W, OC], BF)     # per-window skip   [t, oc]
            st_sb = pers.tile([P, W, EC, NH], BF)   # s_trg per edge slot
            Mw = pers.tile([P, W * cap], BF)      # edge->target one-hot
            Glo = pers.tile([P, W, KLO, 256], BF)  # lo-gathered table rows
            eloidx = pers.tile([P, W * KLO * 8], I16)
            ehiidx = pers.tile([P, W * KHI * 8], I16)
            etcols = pers.tile([P, W * EC], F32)
            kidxs = pers.tile([P, TP // 16], I16)
            etws = pers.tile([1, W * cap], BF)

            nc.sync.dma_start(w1a[:], w1T[0:P, :])
            nc.sync.dma_start(w1b[:], w1T[P:IN, :])
            nc.sync.dma_start(b1s[:], b1[:])
            nc.sync.dma_start(w2s[:], w2[:])
            nc.sync.dma_start(wsks[:], wskT[:])
            nc.sync.dma_start(btrgs[:], btrg[:])
            nc.sync.dma_start(iota4s[:], iota4[:])
            nc.sync.dma_start(iotac[:], iota_c[:])
            nc.sync.dma_start(eloidx[:], eidx_lo[:])
            nc.sync.dma_start(ehiidx[:], eidx_hi[:])
            nc.sync.dma_start(etcols[:], etcol[:])
            nc.sync.dma_start(kidxs[:], kidx[:])
            nc.sync.dma_start(etws[:], etrow[:])

            CH = 2  # 512-node tiles per xT load chunk
            with tc.tile_pool(name="pa", bufs=2) as pa, \
                 tc.tile_pool(name="pax", bufs=2) as pax, \
                 tc.tile_pool(name="pbc", bufs=3) as pbc, \
                 tc.tile_pool(name="pmtw", bufs=2) as pmtw, \
                 tc.tile_pool(name="pghi", bufs=2) as pghi, \
                 tc.tile_pool(name="pe2", bufs=2) as pe2, \
                 tc.tile_pool(name="pko", bufs=1) as pko, \
                 tc.tile_pool(name="pex", bufs=1) as pex, \
                 tc.tile_pool(name="psa", bufs=2, space="PSUM") as psa, \
                 tc.tile_pool(name="psb", bufs=2, space="PSUM") as psb, \
                 tc.tile_pool(name="psc", bufs=1, space="PSUM") as psc, \
                 tc.tile_pool(name="psd", bufs=1, space="PSUM") as psd, \
                 tc.tile_pool(name="pse", bufs=2, space="PSUM") as pse:

                # ---- partition-broadcast of per-slot target cols (Pool) ----
                pbcs = []
                for w in range(W):
                    pbcw = pbc.tile([P, cap], BF, tag="pbcw")
                    nc.gpsimd.partition_broadcast(
                        pbcw[:], etws[0:1, w * cap:(w + 1) * cap])
                    pbcs.append(pbcw)

                # deferred emissions interleaved into phase A slack
                def emit_loop1(w):
                    # s_trg / skip for the window targets
                    stp = psd.tile([P, OC], F32, tag="misc")
                    nc.tensor.matmul(stp[:, 0:NH],
                                     lhsT=hfmt[:, w * P:(w + 1) * P],
                                     rhs=btrgs[:], start=True, stop=True)
                    nc.scalar.activation(strg[:, w * NH:(w + 1) * NH],
                                           stp[:, 0:NH], AF.Copy)
                    skp = psd.tile([P, OC], F32, tag="misc")
                    nc.tensor.matmul(skp[:], lhsT=hfmt[:, w * P:(w + 1) * P],
                                     rhs=wsks[:], start=True, stop=True)
                    nc.scalar.activation(skips[:, w], skp[:], AF.Copy)
                    # target one-hot (col-major) and s_trg edge-slot expansion
                    Mtw = pmtw.tile([P, cap], BF, tag="Mtw")
                    nc.vector.tensor_scalar(Mtw[:], pbcs[w][:], iotac[:], None,
                                            OP.is_equal)
                    stps = psc.tile([P, EC, NH], F32, tag="stps")
                    for j in range(EC):
                        nc.tensor.matmul(
                            stps[:, j, :], lhsT=Mtw[:, j * P:(j + 1) * P],
                            rhs=strg[:, w * NH:(w + 1) * NH],
                            start=True, stop=True)
                    nc.vector.tensor_copy(st_sb[:, w], stps[:])

                def emit_mw(w, j):
                    col = w * EC + j
                    nc.vector.tensor_scalar(
                        Mw[:, col * P:(col + 1) * P], iota4s[:, 0:P],
                        etcols[:, col:col + 1], None, OP.is_equal)

                # schedule: loop1(w) at tile 2+w; Mw slots spread over tiles
                mw_jobs = [(w, j) for w in range(W) for j in range(EC)]
                mw_t0, mw_t1 = 3, NT - 1
                def mw_share(t):
                    if t < mw_t0:
                        return []
                    a = len(mw_jobs) * (t - mw_t0) // (mw_t1 - mw_t0)
                    b = len(mw_jobs) * (t + 1 - mw_t0) // (mw_t1 - mw_t0)
                    return mw_jobs[a:b]

                # ---------------- phase A ----------------
                for t0 in range(0, NT, CH):
                    t1 = min(t0 + CH, NT)
                    wdc = (t1 - t0) * 512
                    slc = slice(t0 * 512, t0 * 512 + wdc)
                    xa = pax.tile([P, CH * 512], BF, tag="xa")
                    nc.sync.dma_start(xa[:, 0:wdc], xT[0:P, slc])
                    xb = pax.tile([P, CH * 512], BF, tag="xb")
                    nc.sync.dma_start(xb[:, 0:wdc], xT[P:IN, slc])
                    for t in range(t0, t1):
                        o = (t - t0) * 512
                        hps = psa.tile([P, 512], F32, tag="hps")
                        nc.tensor.matmul(hps[:], lhsT=w1a[:],
                                         rhs=xa[:, o:o + 512],
                                         start=True, stop=False)
                        nc.tensor.matmul(hps[:], lhsT=w1b[:],
                                         rhs=xb[:, o:o + 512],
                                         start=False, stop=True)
                        hsb = pa.tile([P, 512], F32R, tag="hsb")
                        nc.scalar.activation(hsb[:], hps[:], AF.Relu,
                                             bias=b1s[:])
                        if t * 512 < TP:
                            w0 = t * 512
                            w1_ = min(TP, (t + 1) * 512)
                            nc.scalar.activation(hfmt[:, w0:w1_],
                                                 hps[:, 0:(w1_ - w0)], AF.Relu,
                                                 bias=b1s[:])
                        stg = pa.tile([P, 4, 256], BF, tag="stg")
                        for half in range(2):
                            p2 = psb.tile([P, 2, 256], F32, tag="p2")
                            for jj in range(2):
                                j = half * 2 + jj
                                nc.tensor.matmul(
                                    p2[:, jj, :],
                                    lhsT=hsb[:, j * P:(j + 1) * P],
                                    rhs=w2s[:], start=True, stop=True)
                            sgh = stg[:, half * 2:half * 2 + 2, :]
                            if half == 0:
                                nc.scalar.activation(sgh[:, :, 0:OC + NH],
                                                     p2[:, :, 0:OC + NH],
                                                     AF.Copy)
                            else:
                                nc.vector.tensor_copy(sgh[:, :, 0:OC + NH],
                                                      p2[:, :, 0:OC + NH])
                        r0 = t * 512
                        nc.sync.dma_start(
                            tabA[r0:r0 + 512, :].rearrange(
                                "(j p) f -> p j f", p=P), stg[:])
                        # interleaved loop-1 / mask emissions
                        if 8 <= t < 8 + 2 * W and (t - 8) % 2 == 0:
                            emit_loop1((t - 8) // 2)
                        for (w_, j_) in mw_share(t):
                            emit_mw(w_, j_)

                # ---------------- gathers ----------------
                # lo gathers read only rows [0, B1): they fire as soon as
                # the lo part of the table is written (overlaps phase A)
                for w in range(W):
                    nc.gpsimd.dma_gather(
                        Glo[:, w], tabA[0:B1],
                        eloidx[:, w * KLO * 8:(w + 1) * KLO * 8],
                        KLO * P, KLO * P, 256, single_packet=False)
                ghis = []
                for w in range(W):
                    G = pghi.tile([P, KHI, 256], BF, tag="G")
                    nc.gpsimd.dma_gather(
                        G[:], tabA[:],
                        ehiidx[:, w * KHI * 8:(w + 1) * KHI * 8],
                        KHI * P, KHI * P, 256, single_packet=False)
                    ghis.append(G)

                # ---------------- loop 2: per-window edge pipeline ----------
                def finalize(w, segp):
                    den = pe2.tile([P, NH], F32, tag="den")
                    nc.vector.tensor_scalar_add(den[:], segp[:, OC:OC + NH],
                                                EPS)
                    rec = pe2.tile([P, NH], F32, tag="rec")
                    nc.vector.reciprocal(rec[:], den[:])
                    z = pe2.tile([P, OC], F32, tag="z")
                    recb = rec[:].broadcast_to([P, NH, HD])
                    nc.vector.tensor_tensor(
                        z[:].rearrange("p (a d) -> p a d", d=HD),
                        segp[:, 0:OC].rearrange("p (a d) -> p a d", d=HD),
                        recb, OP.mult)
                    nc.vector.tensor_add(z[:], z[:], skips[:, w])
                    # elu: (max(z,0)-1) + exp(min(z,0))
                    am = pe2.tile([P, OC], F32, tag="am")
                    nc.vector.tensor_scalar(am[:], z[:], 0.0, -1.0, OP.max,
                                            OP.add)
                    bm = pe2.tile([P, OC], F32, tag="bm")
                    nc.gpsimd.tensor_scalar(bm[:], z[:], 0.0, None, OP.min)
                    eb = pe2.tile([P, OC], F32, tag="eb")
                    nc.scalar.activation(eb[:], bm[:], AF.Exp)
                    fo = pe2.tile([P, OC], BF, tag="fo")
                    nc.vector.tensor_add(fo[:], am[:], eb[:])
                    nc.sync.dma_start(outT[w * P:(w + 1) * P, :], fo[:])

                pending = None
                for w in range(W):
                    G = ghis[w]
                    # scores = s_src(gathered) + s_trg(expanded)
                    sc = pe2.tile([P, EC, NH], F32, tag="sc")
                    glo_ss = Glo[:, w, :, OC:OC + NH]
                    nc.vector.tensor_tensor(sc[:, 0:KLO], st_sb[:, w, 0:KLO],
                                            glo_ss, OP.add)
                    if KHI:
                        nc.vector.tensor_tensor(sc[:, KLO:EC],
                                                st_sb[:, w, KLO:EC],
                                                G[:, :, OC:OC + NH], OP.add)
                    # e = exp(leakyrelu(s)) = max(exp(s), exp(0.2 s))
                    e1 = pe2.tile([P, EC, NH], BF, tag="e1")
                    nc.scalar.activation(e1[:], sc[:], AF.Exp)
                    e2 = pe2.tile([P, EC, NH], BF, tag="e2")
                    nc.scalar.activation(e2[:], sc[:], AF.Exp, scale=0.2)
                    emax = pe2.tile([P, EC, NH], BF, tag="emax")
                    nc.vector.tensor_max(emax[:], e1[:], e2[:])
                    Wv = pe2.tile([P, EC, 136], BF, tag="Wv")
                    nc.vector.tensor_copy(Wv[:, :, OC:OC + NH], emax[:])
                    emb = emax[:].broadcast_to([P, EC, NH, HD])
                    wv4 = Wv[:, :, 0:OC].rearrange("p j (a d) -> p j a d", d=HD)
                    if w % 2 == 0:
                        # expand e per-head on ACT (broadcast read), then a
                        # fully packed bf16 multiply on DVE (2x mode)
                        eex = pex.tile([P, EC, OC], BF, tag="eex")
                        nc.scalar.activation(
                            eex[:].rearrange("p j (a d) -> p j a d", d=HD),
                            emb, AF.Copy)
                        nc.vector.tensor_tensor(Wv[:, 0:KLO, 0:OC],
                                                Glo[:, w, :, 0:OC],
                                                eex[:, 0:KLO], OP.mult)
                        nc.vector.tensor_tensor(Wv[:, KLO:EC, 0:OC],
                                                G[:, :, 0:OC],
                                                eex[:, KLO:EC], OP.mult)
                    else:
                        # direct broadcast multiply on DVE (1x, no expand)
                        glo4 = Glo[:, w, :, 0:OC].rearrange(
                            "p j (a d) -> p j a d", d=HD)
                        ghi4 = G[:, :, 0:OC].rearrange(
                            "p j (a d) -> p j a d", d=HD)
                        nc.vector.tensor_tensor(wv4[:, 0:KLO], glo4,
                                                emb[:, 0:KLO], OP.mult)
                        nc.vector.tensor_tensor(wv4[:, KLO:EC], ghi4,
                                                emb[:, KLO:EC], OP.mult)

                    segp = pse.tile([P, 136], F32, tag="segp")
                    for j in range(EC):
                        nc.tensor.matmul(segp[:],
                                         lhsT=Mw[:, (w * EC + j) * P:
                                                 (w * EC + j + 1) * P],
                                         rhs=Wv[:, j, :], start=(j == 0),
                                         stop=(j == EC - 1))
                    if pending is not None:
                        finalize(*pending)
                    pending = (w, segp)
                if pending is not None:
                    finalize(*pending)

                # final k-row gather
                ko = pko.tile([P, TP // P, OC], BF, tag="ko")
                nc.gpsimd.dma_gather(ko[:], outT[:], kidxs[:], TP, TP, OC,
                                     single_packet=False)
                nc.sync.dma_start(
                    out[:].rearrange("(j p) f -> p j f", p=P), ko[:])

    nc.compile()
    _BUILD_CACHE[key] = nc
    return nc


# ----------------------------------------------------------------------------
# entry point
# ----------------------------------------------------------------------------

def kernel(x, adj0, index0, w_in, b_in, w_proj, a_src, a_trg, w_skip):
    from concourse.bass_utils import run_bass_kernel_spmd

    per_core, NPAD, EC, KLO, B1 = plan(x, adj0, index0)
    wts = make_weights(w_in, b_in, w_proj, a_src, a_trg, w_skip)
    nc = build(NPAD, EC, KLO, B1)

    in_maps = []
    for c in range(CORES):
        m = dict(wts)
        m.update(per_core[c])
        in_maps.append(m)

    res = run_bass_kernel_spmd(nc, in_maps, core_ids=list(range(CORES)))
    outs = [r["out"][:KC] for r in res.results]
    return np.concatenate(outs, axis=0).astype(np.float32)
